# revision 4
# baseline (speedup 1.0000x reference)
"""Trainium2 Bass kernel for nn_ExplicitGeometricAugmentor.

Data-parallel over batch: 32 images -> 8 cores x 4 images. No collectives.

Per-core layout: feature-major activations [dim, tok] (tok = 4*196 = 784).
All matmuls fp32r (full PE rate at moving-free >= 256). LayerNorm is folded
into the adjacent matmuls via augmented contraction rows; only v (not q,k)
of the qkv projection is computed since the RBF attention ignores q,k.
"""
import math
import numpy as np

import concourse.mybir as mybir
import concourse.tile as tile
from concourse import bacc
from concourse.bass_utils import run_bass_kernel_spmd

F32 = mybir.dt.float32
F32R = mybir.dt.float32r
AF = mybir.ActivationFunctionType
ALU = mybir.AluOpType

IMG = 224; PCH = 16; C = 3
G = IMG // PCH; N = G * G          # 14, 196
DIM = 768; DEPTH = 6; MLP = 3072
INNER = 768; PD = 768
B = 32
NCORES = 8
BPC = B // NCORES                  # 4 images per core
TOK = BPC * N                      # 784 tokens per core
TT = 98                            # token tile (196 = 2*98 -> image == 2 tiles)
NTT = TOK // TT                    # 8 token tiles
KT = DIM // 128                    # 6 feature k-tiles
KM = MLP // 128                    # 24 mlp k-tiles
CH = 392                           # token chunk (free dim of most matmuls)
NCH = TOK // CH                    # 2
APAD = 256                         # attention moving-dim pad (fp32r full rate)
EPS = 1e-5
JG = 2                             # ffn j-group size (wf1 residency granularity)
NG = KM // JG                      # 6 groups

_STATE: dict = {}


# ----------------------------------------------------------------- bass build
def _build():
    nc = bacc.Bacc("TRN2", debug=False, target_bir_lowering=False)
    d = {}
    d["xp"] = nc.dram_tensor("xp", (DIM, TOK), F32R, kind="ExternalInput").ap()
    d["att"] = nc.dram_tensor("att", (TT, NTT * APAD), F32R, kind="ExternalInput").ap()
    d["wp"] = nc.dram_tensor("wp", (DIM + 1, DIM), F32R, kind="ExternalInput").ap()
    d["wv"] = nc.dram_tensor("wv", (DEPTH, DIM + 2, DIM), F32R, kind="ExternalInput").ap()
    d["wo"] = nc.dram_tensor("wo", (DEPTH, DIM + 1, DIM), F32R, kind="ExternalInput").ap()
    d["wf1"] = nc.dram_tensor("wf1", (DEPTH, DIM + 2, MLP), F32R, kind="ExternalInput").ap()
    d["wf2"] = nc.dram_tensor("wf2", (DEPTH, 128, KM * DIM), F32R, kind="ExternalInput").ap()
    d["bf2"] = nc.dram_tensor("bf2", (DEPTH, 128, KT), F32, kind="ExternalInput").ap()
    d["we"] = nc.dram_tensor("we", (DIM + 1, PD), F32R, kind="ExternalInput").ap()
    d["onec"] = nc.dram_tensor("onec", (128, 1), F32R, kind="ExternalInput").ap()
    d["oner"] = nc.dram_tensor("oner", (1, TOK), F32R, kind="ExternalInput").ap()
    d["out"] = nc.dram_tensor("out", (PD, TOK), F32, kind="ExternalOutput").ap()
    d["scrb"] = nc.dram_tensor("scrb", (DEPTH, TOK), F32, kind="Internal").ap()

    with tile.TileContext(nc) as tc:
        with tc.tile_pool(name="sb", bufs=1) as sb, \
             tc.tile_pool(name="psA", bufs=2, space="PSUM") as psA, \
             tc.tile_pool(name="psB", bufs=6, space="PSUM") as psB:
            _emit(nc, tc, d, sb, psA, psB)
    nc.compile()
    return nc


def _emit(nc, tc, d, sb, psA, psB):
    chsl = [slice(ch * CH, (ch + 1) * CH) for ch in range(NCH)]

    # ---- persistent constants
    ATT = sb.tile([TT, NTT * APAD], F32R, tag="attc", name="ATT")
    nc.sync.dma_start(out=ATT, in_=d["att"])
    ones_col = sb.tile([128, 1], F32R, tag="onec", name="ones_col")
    nc.sync.dma_start(out=ones_col, in_=d["onec"])
    oner_sb = sb.tile([1, TOK], F32R, tag="onerr", name="oner_sb")
    nc.sync.dma_start(out=oner_sb, in_=d["oner"])
    id1 = sb.tile([1, 1], F32, tag="id1", name="id1")
    nc.vector.memset(id1, 1.0)
    epsb = sb.tile([1, 1], F32, tag="epsb", name="epsb")
    nc.vector.memset(epsb, EPS)

    def rowf(nm):
        return sb.tile([1, TOK], F32, tag="rwf", bufs=2, name=nm)

    def rowr(nm):
        return sb.tile([1, TOK], F32R, tag="rwr", bufs=2, name=nm)

    # ---- dense projection out[dp 128-tiles, tok] = W.T @ src (+ bias row via ones aug)
    # wsrc rows: [0:DIM]=W, row DIM = bias. src: list of 6 [128, TOK] f32r tiles.
    def dense_proj(wsrc, src, evict, nm):
        for dp in range(KT):
            wt = []
            for k in range(KT):
                w = sb.tile([128, 128], F32R, tag="wod", bufs=7, name=f"{nm}w{dp}_{k}")
                nc.sync.dma_start(out=w, in_=wsrc[k * 128:(k + 1) * 128,
                                                 dp * 128:(dp + 1) * 128])
                wt.append(w)
            wb = sb.tile([1, 128], F32R, tag="wob", bufs=2, name=f"{nm}b{dp}")
            nc.sync.dma_start(out=wb, in_=wsrc[DIM:DIM + 1, dp * 128:(dp + 1) * 128])
            for ch in range(NCH):
                pm = psA.tile([128, CH], F32, tag="mm", name=f"{nm}p{dp}_{ch}")
                for k in range(KT):
                    nc.tensor.matmul(pm, wt[k], src[k][:, chsl[ch]],
                                     start=(k == 0), stop=False)
                nc.tensor.matmul(pm, wb, oner_sb[:, chsl[ch]], start=False, stop=True)
                evict(dp, ch, pm)

    # ---- LN stats for 6 [128, TOK] f32r tiles -> (statsA[2,TOK] f32r rows, rstd f32 row)
    def ln_stats(src, nm, want_col, want_msr):
        mu = rowr(f"{nm}mu")
        e2 = rowf(f"{nm}e2")
        for ch in range(NCH):
            pS = psA.tile([1, CH], F32, tag="mm", name=f"{nm}pS{ch}")
            pQ = psA.tile([1, CH], F32, tag="mm", name=f"{nm}pQ{ch}")
            for k in range(KT):
                nc.tensor.matmul(pS, ones_col, src[k][:, chsl[ch]],
                                 start=(k == 0), stop=(k == KT - 1))
            for k in range(KT):
                s = sb.tile([128, CH], F32R, tag="scr", bufs=3, name=f"{nm}sq{ch}_{k}")
                nc.vector.tensor_mul(out=s, in0=src[k][:, chsl[ch]],
                                     in1=src[k][:, chsl[ch]])
                nc.tensor.matmul(pQ, ones_col, s,
                                 start=(k == 0), stop=(k == KT - 1))
            nc.vector.tensor_scalar_mul(mu[:, chsl[ch]], pS, 1.0 / DIM)
            nc.vector.tensor_scalar_mul(e2[:, chsl[ch]], pQ, 1.0 / DIM)
        var = rowf(f"{nm}var")
        nc.vector.tensor_mul(out=var, in0=mu, in1=mu)
        nc.vector.tensor_sub(out=var, in0=e2, in1=var)
        std = rowr(f"{nm}std")
        nc.scalar.activation(std, var, AF.Sqrt, bias=epsb)
        rstd = rowf(f"{nm}rstd")
        nc.vector.reciprocal(out=rstd, in_=std)
        stats = sb.tile([2, TOK], F32R, tag="sta", bufs=1, name=f"{nm}stats")
        if want_msr:
            msr = rowr(f"{nm}msr")
            nc.vector.tensor_mul(out=msr, in0=mu, in1=rstd)
            nc.sync.dma_start(out=stats[0:1, :], in_=msr)
            nc.sync.dma_start(out=stats[1:2, :], in_=d["oner"])
        else:
            nc.sync.dma_start(out=stats[0:1, :], in_=mu)
            nc.sync.dma_start(out=stats[1:2, :], in_=std)
        rcol = None
        if want_col:
            rcol = sb.tile([TT, NTT], F32, tag="rcol", bufs=1, name=f"{nm}rcol")
            pc = psA.tile([TT, NTT], F32, tag="mm", name=f"{nm}pcol")
            for m in range(NTT):
                nc.tensor.transpose(pc[:, m:m + 1],
                                    rstd.bitcast(F32)[:, m * TT:(m + 1) * TT], id1)
            nc.vector.tensor_copy(out=rcol, in_=pc)
        return stats, rstd, rcol

    # ---- embed: X = Wp.T @ xp + bp
    X = [None] * KT
    XP = []
    for k in range(KT):
        t = sb.tile([128, TOK], F32R, tag=f"xh{k}", name=f"XP{k}")
        nc.sync.dma_start(out=t, in_=d["xp"][k * 128:(k + 1) * 128, :])
        XP.append(t)

    def ev_embed(dp, ch, pm):
        if ch == 0:
            X[dp] = sb.tile([128, TOK], F32R, tag=f"x{dp}", name=f"X0_{dp}")
        nc.vector.tensor_copy(out=X[dp][:, chsl[ch]], in_=pm)

    dense_proj(d["wp"], XP, ev_embed, "emb")

    # ---- transformer layers
    for l in range(DEPTH):
        statsA, _rstd1, rcol = ln_stats(X, f"l{l}a", want_col=True, want_msr=False)

        # v-projection (activation-stationary -> token-major v), LN1 folded.
        VT = [None] * NTT
        wvf = []
        for k in range(KT):
            w = sb.tile([128, DIM], F32R, tag="wv", bufs=6, name=f"l{l}wv{k}")
            nc.sync.dma_start(out=w, in_=d["wv"][l, k * 128:(k + 1) * 128, :])
            wvf.append(w)
        wvb = sb.tile([2, DIM], F32R, tag="wvb", bufs=1, name=f"l{l}wvb")
        nc.sync.dma_start(out=wvb, in_=d["wv"][l, DIM:DIM + 2, :])
        # v-proj fused with attention per image (VT slots recycle promptly)
        OT = []
        for f in range(KT):
            o = sb.tile([128, TOK], F32R, tag=f"ot{f}", name=f"l{l}ot{f}")
            OT.append(o)
        for m in range(NTT):
            msl = slice(m * TT, (m + 1) * TT)
            VT[m] = sb.tile([TT, INNER], F32R, tag="vt", bufs=3, name=f"l{l}v{m}")
            for n2 in range(2):
                nsl = slice(n2 * 384, (n2 + 1) * 384)
                pv = psA.tile([TT, 384], F32, tag="mm", name=f"l{l}pv{m}_{n2}")
                for k in range(KT):
                    nc.tensor.matmul(pv, X[k][:, msl], wvf[k][:, nsl],
                                     start=(k == 0), stop=False)
                nc.tensor.matmul(pv, statsA[:, msl], wvb[:, nsl], start=False, stop=True)
                nc.vector.tensor_scalar_mul(VT[m][:, nsl], pv, rcol[:, m:m + 1])
            if m % 2 == 1:
                b = m // 2
                for f in range(KT):
                    po = psA.tile([128, APAD], F32, tag="mm", name=f"l{l}po{b}_{f}")
                    for h in range(2):
                        mm = 2 * b + h
                        nc.tensor.matmul(po, VT[mm][:, f * 128:(f + 1) * 128],
                                         ATT[:, mm * APAD:(mm + 1) * APAD],
                                         start=(h == 0), stop=(h == 1))
                    nc.scalar.activation(OT[f][:, b * N:(b + 1) * N], po[:, 0:N], AF.Copy)

        # out-projection + bout
        X2 = [None] * KT

        def ev_x2(dp, ch, pm, l=l):
            if ch == 0:
                X2[dp] = sb.tile([128, TOK], F32R, tag=f"x2_{dp}", name=f"l{l}x2_{dp}")
            nc.vector.tensor_copy(out=X2[dp][:, chsl[ch]], in_=pm)

        dense_proj(d["wo"][l], OT, ev_x2, f"l{l}o")

        # LN2 -> statsB rows [msr2, ones]; P2 broadcast; XH = X2 * rstd2
        statsB, rstd2, _ = ln_stats(X2, f"l{l}b", want_col=False, want_msr=True)
        nc.sync.dma_start(out=d["scrb"][l], in_=rstd2)
        P2 = sb.tile([128, TOK], F32, tag="p2", name=f"l{l}p2")
        nc.sync.dma_start(
            out=P2, in_=d["scrb"][l:l + 1, :].to_broadcast([128, TOK]))
        XH = []
        for k in range(KT):
            t = sb.tile([128, TOK], F32R, tag=f"xh{k}", name=f"l{l}xh{k}")
            nc.vector.tensor_mul(out=t, in0=X2[k], in1=P2)
            XH.append(t)

        # bff2 column
        bf2c = sb.tile([128, KT], F32, tag="bf2c", bufs=1, name=f"l{l}bf2c")
        nc.sync.dma_start(out=bf2c, in_=d["bf2"][l])

        # FFN: ch-outer, full 24-j psum accumulation (6 acc banks + 2 ffn1 banks)
        for ch in range(NCH):
            acc = [psB.tile([128, CH], F32, tag="acc", name=f"l{l}acc{ch}_{dp}")
                   for dp in range(KT)]
            for g in range(NG):
                gsl = slice(g * JG * 128, (g + 1) * JG * 128)
                w1g = []
                for k in range(KT):
                    w = sb.tile([128, JG * 128], F32R, tag="wf1", bufs=12,
                                name=f"l{l}f1_{ch}_{g}_{k}")
                    nc.sync.dma_start(out=w, in_=d["wf1"][l, k * 128:(k + 1) * 128, gsl])
                    w1g.append(w)
                w1b = sb.tile([2, JG * 128], F32R, tag="wf1b", bufs=1,
                              name=f"l{l}f1b{ch}_{g}")
                nc.sync.dma_start(out=w1b, in_=d["wf1"][l, DIM:DIM + 2, gsl])
                for jj in range(JG):
                    j = g * JG + jj
                    jsl = slice(jj * 128, (jj + 1) * 128)
                    w2 = sb.tile([128, DIM], F32R, tag="wf2", bufs=3,
                                 name=f"l{l}f2_{ch}_{j}")
                    nc.sync.dma_start(out=w2, in_=d["wf2"][l][:, j * DIM:(j + 1) * DIM])
                    ph = psA.tile([128, CH], F32, tag="mm", name=f"l{l}ph{ch}_{j}")
                    for k in range(KT):
                        nc.tensor.matmul(ph, w1g[k][:, jsl], XH[k][:, chsl[ch]],
                                         start=(k == 0), stop=False)
                    nc.tensor.matmul(ph, w1b[:, jsl], statsB[:, chsl[ch]],
                                     start=False, stop=True)
                    h1 = sb.tile([128, CH], F32R, tag="h1", bufs=2, name=f"l{l}h1_{ch}_{j}")
                    nc.scalar.activation(h1, ph, AF.Gelu)
                    for dp in range(KT):
                        nc.tensor.matmul(acc[dp], w2[:, dp * 128:(dp + 1) * 128], h1,
                                         start=(j == 0), stop=(j == KM - 1))
            # evict: X_next = acc + bff2 + X2  (residual)
            for dp in range(KT):
                if ch == 0:
                    X[dp] = sb.tile([128, TOK], F32R, tag=f"x{dp}", name=f"l{l}xn{dp}")
                nc.vector.scalar_tensor_tensor(
                    out=X[dp][:, chsl[ch]], in0=acc[dp], scalar=bf2c[:, dp:dp + 1],
                    in1=X2[dp][:, chsl[ch]], op0=ALU.add, op1=ALU.add)

    # ---- head: out = Wep.T @ x + bep
    def ev_out(dp, ch, pm):
        t = sb.tile([128, CH], F32, tag="scr", bufs=3, name=f"hd{dp}_{ch}")
        nc.vector.tensor_copy(out=t, in_=pm)
        nc.sync.dma_start(out=d["out"][dp * 128:(dp + 1) * 128, chsl[ch]], in_=t)

    dense_proj(d["we"], X, ev_out, "hd")


# ------------------------------------------------------------------ host prep
def _softplus(x):
    return np.logaddexp(x, 0.0)


def _attention(A_noise, b_noise, A_mean, b_mean, A_std, b_std, band_width):
    m, n = np.meshgrid(np.arange(G), np.arange(G))
    Xc = np.stack((m, n), axis=-1).reshape(-1, 2).astype(np.float64)
    A = A_mean.astype(np.float64) + _softplus(A_std.astype(np.float64)) * A_noise.astype(np.float64)
    bt = b_mean.astype(np.float64) + _softplus(b_std.astype(np.float64)) * b_noise.astype(np.float64)
    Xt = np.einsum('pd,bdo->bpo', Xc, A) + bt[:, None, :]
    diff = Xt[:, None, :, :] - Xc[None, :, None, :]
    dots = -np.sum(diff * diff, axis=-1) / float(band_width)
    dots -= dots.max(axis=-1, keepdims=True)
    e = np.exp(dots)
    attn = e / e.sum(axis=-1, keepdims=True)
    return attn.astype(np.float32)            # (B, N, N)


def _prep(inputs):
    f = np.float32
    w = {}
    Wp = inputs["Wp"].astype(f); bp = inputs["bp"].astype(f)
    w["wp"] = np.ascontiguousarray(np.vstack([Wp, bp[None, :]]))
    Wep = inputs["Wep"].astype(f); bep = inputs["bep"].astype(f)
    w["we"] = np.ascontiguousarray(np.vstack([Wep, bep[None, :]]))
    wv = np.zeros((DEPTH, DIM + 2, DIM), f)
    wo = np.zeros((DEPTH, DIM + 1, DIM), f)
    wf1 = np.zeros((DEPTH, DIM + 2, MLP), f)
    wf2 = np.zeros((DEPTH, 128, KM * DIM), f)
    bf2 = np.zeros((DEPTH, 128, KT), f)
    for l in range(DEPTH):
        g1 = inputs["ln1_g"][l].astype(f); b1 = inputs["ln1_b"][l].astype(f)
        Wv = inputs["Wqkv"][l][:, 2 * INNER:].astype(f)
        wv[l, :DIM] = g1[:, None] * Wv
        wv[l, DIM] = -(g1 @ Wv)
        wv[l, DIM + 1] = b1 @ Wv
        wo[l, :DIM] = inputs["Wout"][l].astype(f)
        wo[l, DIM] = inputs["bout"][l].astype(f)
        g2 = inputs["ln2_g"][l].astype(f); b2 = inputs["ln2_b"][l].astype(f)
        Wf1 = inputs["Wff1"][l].astype(f)
        wf1[l, :DIM] = g2[:, None] * Wf1
        wf1[l, DIM] = -(g2 @ Wf1)
        wf1[l, DIM + 1] = b2 @ Wf1 + inputs["bff1"][l].astype(f)
        wf2[l] = inputs["Wff2"][l].astype(f).reshape(KM, 128, DIM).transpose(1, 0, 2).reshape(128, KM * DIM)
        bf2[l] = inputs["bff2"][l].astype(f).reshape(KT, 128).T
    w["wv"] = wv; w["wo"] = wo; w["wf1"] = wf1; w["wf2"] = wf2; w["bf2"] = bf2
    w["onec"] = np.ones((128, 1), f)
    w["oner"] = np.ones((1, TOK), f)
    return w


def _prep_percore(inputs):
    f = np.float32
    img = inputs["img"].astype(f)
    patches = img.reshape(B, C, G, PCH, G, PCH).transpose(0, 2, 4, 3, 5, 1).reshape(B, N, PD)
    attn = _attention(inputs["A_noise"], inputs["b_noise"], inputs["A_mean"],
                      inputs["b_mean"], inputs["A_std"], inputs["b_std"],
                      inputs["band_width"])
    per = []
    for c in range(NCORES):
        pc = patches[c * BPC:(c + 1) * BPC].reshape(TOK, PD)
        xp = np.ascontiguousarray(pc.T)
        ac = attn[c * BPC:(c + 1) * BPC]                      # (4, N, N) [b, i, j]
        at = np.zeros((TT, NTT, APAD), f)
        at[:, :, :N] = ac.transpose(0, 2, 1).reshape(BPC, 2, TT, N).transpose(2, 0, 1, 3).reshape(TT, NTT, N)
        per.append({"xp": xp, "att": np.ascontiguousarray(at.reshape(TT, NTT * APAD))})
    return per


def _unpack(results):
    outs = []
    for c in range(NCORES):
        o = results[c]["out"]                                # (PD, TOK)
        x = o.T.reshape(BPC, G, G, PCH, PCH, C).transpose(0, 5, 1, 3, 2, 4)
        outs.append(x.reshape(BPC, C, IMG, IMG))
    return np.concatenate(outs, axis=0)


# ------------------------------------------------------------------- kernel()
def kernel(**inputs) -> np.ndarray:
    if "nc" not in _STATE:
        _STATE["nc"] = _build()
    if "w" not in _STATE:
        _STATE["w"] = _prep(inputs)
    per = _prep_percore(inputs)
    in_maps = [{**per[c], **_STATE["w"]} for c in range(NCORES)]
    res = run_bass_kernel_spmd(_STATE["nc"], in_maps, core_ids=list(range(NCORES)))
    return _unpack(res.results)


# revision 7
# speedup vs baseline: 1.2286x; 1.2286x over previous
"""Trainium2 Bass kernel for nn_ExplicitGeometricAugmentor.

Data-parallel over batch: 32 images -> 8 cores x 4 images. No collectives.

Per-core layout: feature-major activations [dim, tok] (tok = 4*196 = 784).
All matmuls fp32r (full PE rate at moving-free >= 256). LayerNorm is folded
into the adjacent matmuls via augmented contraction rows; only v (not q,k)
of the qkv projection is computed since the RBF attention ignores q,k.
"""
import math
import numpy as np

import concourse.mybir as mybir
import concourse.tile as tile
from concourse import bacc
from concourse.bass_utils import run_bass_kernel_spmd

F32 = mybir.dt.float32
F32R = mybir.dt.float32r
AF = mybir.ActivationFunctionType
ALU = mybir.AluOpType

import os
IMG = 224; PCH = 16; C = 3
G = IMG // PCH; N = G * G          # 14, 196
DIM = 768; DEPTH = int(os.environ.get("KD", "6")); MLP = 3072
INNER = 768; PD = 768
B = 32
NCORES = 8
BPC = B // NCORES                  # 4 images per core
TOK = BPC * N                      # 784 tokens per core
TT = 98                            # token tile (196 = 2*98 -> image == 2 tiles)
NTT = TOK // TT                    # 8 token tiles
KT = DIM // 128                    # 6 feature k-tiles
KM = MLP // 128                    # 24 mlp k-tiles
CH = 392                           # token chunk (free dim of most matmuls)
NCH = TOK // CH                    # 2
APAD = 256                         # attention moving-dim pad (fp32r full rate)
EPS = 1e-5
JG = 3                             # ffn j-group size (weight residency granularity)
NG = KM // JG                      # 8 groups

_STATE: dict = {}


# ----------------------------------------------------------------- bass build
def _build():
    nc = bacc.Bacc("TRN2", debug=False, target_bir_lowering=False)
    d = {}
    d["xp"] = nc.dram_tensor("xp", (DIM, TOK), F32R, kind="ExternalInput").ap()
    d["att"] = nc.dram_tensor("att", (TT, NTT * APAD), F32R, kind="ExternalInput").ap()
    d["wp"] = nc.dram_tensor("wp", (DIM + 1, DIM), F32R, kind="ExternalInput").ap()
    d["wv"] = nc.dram_tensor("wv", (DEPTH, DIM + 2, DIM), F32R, kind="ExternalInput").ap()
    d["wo"] = nc.dram_tensor("wo", (DEPTH, DIM + 1, DIM), F32R, kind="ExternalInput").ap()
    d["wf1"] = nc.dram_tensor("wf1", (DEPTH, DIM + 2, MLP), F32R, kind="ExternalInput").ap()
    d["wf2"] = nc.dram_tensor("wf2", (DEPTH, 128, KM * DIM), F32R, kind="ExternalInput").ap()
    d["bf2"] = nc.dram_tensor("bf2", (DEPTH, 128, KT), F32, kind="ExternalInput").ap()
    d["we"] = nc.dram_tensor("we", (DIM + 1, PD), F32R, kind="ExternalInput").ap()
    d["onec"] = nc.dram_tensor("onec", (128, 1), F32R, kind="ExternalInput").ap()
    d["oner"] = nc.dram_tensor("oner", (1, TOK), F32R, kind="ExternalInput").ap()
    d["out"] = nc.dram_tensor("out", (PD, TOK), F32, kind="ExternalOutput").ap()
    d["scrb"] = nc.dram_tensor("scrb", (DEPTH, TOK), F32, kind="Internal").ap()

    with tile.TileContext(nc) as tc:
        with tc.tile_pool(name="sb", bufs=1) as sb, \
             tc.tile_pool(name="psA", bufs=2, space="PSUM") as psA, \
             tc.tile_pool(name="psB", bufs=6, space="PSUM") as psB:
            _emit(nc, tc, d, sb, psA, psB)
    nc.compile()
    return nc


def _emit(nc, tc, d, sb, psA, psB):
    chsl = [slice(ch * CH, (ch + 1) * CH) for ch in range(NCH)]
    _mmctr = [0]

    def mmtile(shape, name):
        # round-robin psum slots: 2 in psA ("mm") + 6 in psB ("acc") = 8 banks
        _mmctr[0] += 1
        if _mmctr[0] % 4 == 0:
            return psA.tile(shape, F32, tag="mm", name=name)
        return psB.tile(shape, F32, tag="acc", name=name)

    # ---- persistent constants
    ATT = sb.tile([TT, NTT * APAD], F32R, tag="attc", name="ATT")
    nc.sync.dma_start(out=ATT, in_=d["att"])
    ones_col = sb.tile([128, 1], F32R, tag="onec", name="ones_col")
    nc.sync.dma_start(out=ones_col, in_=d["onec"])
    oner_sb = sb.tile([1, TOK], F32R, tag="onerr", name="oner_sb")
    nc.sync.dma_start(out=oner_sb, in_=d["oner"])
    id1 = sb.tile([1, 1], F32, tag="id1", name="id1")
    nc.vector.memset(id1, 1.0)
    epsb = sb.tile([1, 1], F32, tag="epsb", name="epsb")
    nc.vector.memset(epsb, EPS)

    def rowf(nm):
        return sb.tile([1, TOK], F32, tag="rwf", bufs=3, name=nm)

    def rowr(nm):
        return sb.tile([1, TOK], F32R, tag="rwr", bufs=3, name=nm)

    # ---- dense projection out[dp 128-tiles, tok] = W.T @ src (+ bias row via ones aug)
    # wsrc rows: [0:DIM]=W, row DIM = bias. src: list of 6 [128, TOK] f32r tiles.
    def dense_proj(wsrc, src, evict, nm):
        for dp in range(KT):
            wt = []
            for k in range(KT):
                w = sb.tile([128, 128], F32R, tag="wod", bufs=12, name=f"{nm}w{dp}_{k}")
                nc.sync.dma_start(out=w, in_=wsrc[k * 128:(k + 1) * 128,
                                                 dp * 128:(dp + 1) * 128])
                wt.append(w)
            wb = sb.tile([1, 128], F32R, tag="wob", bufs=2, name=f"{nm}b{dp}")
            nc.sync.dma_start(out=wb, in_=wsrc[DIM:DIM + 1, dp * 128:(dp + 1) * 128])
            for ch in range(NCH):
                pm = mmtile([128, CH], f"{nm}p{dp}_{ch}")
                for k in range(KT):
                    nc.tensor.matmul(pm, wt[k], src[k][:, chsl[ch]],
                                     start=(k == 0), stop=False)
                nc.tensor.matmul(pm, wb, oner_sb[:, chsl[ch]], start=False, stop=True)
                evict(dp, ch, pm)

    # ---- LN stats for 6 [128, TOK] f32r tiles -> (statsA[2,TOK] f32r rows, rstd f32 row)
    def ln_stats(src, nm, want_col, want_msr):
        mu = rowr(f"{nm}mu")
        e2 = rowf(f"{nm}e2")
        for ch in range(NCH):
            pS = mmtile([1, CH], f"{nm}pS{ch}")
            pQ = mmtile([1, CH], f"{nm}pQ{ch}")
            for k in range(KT):
                nc.tensor.matmul(pS, ones_col, src[k][:, chsl[ch]],
                                 start=(k == 0), stop=(k == KT - 1))
            for k in range(KT):
                s = sb.tile([128, CH], F32R, tag="scr", bufs=3, name=f"{nm}sq{ch}_{k}")
                nc.vector.tensor_mul(out=s, in0=src[k][:, chsl[ch]],
                                     in1=src[k][:, chsl[ch]])
                nc.tensor.matmul(pQ, ones_col, s,
                                 start=(k == 0), stop=(k == KT - 1))
            nc.vector.tensor_scalar_mul(mu[:, chsl[ch]], pS, 1.0 / DIM)
            nc.vector.tensor_scalar_mul(e2[:, chsl[ch]], pQ, 1.0 / DIM)
        var = rowf(f"{nm}var")
        nc.vector.tensor_mul(out=var, in0=mu, in1=mu)
        nc.vector.tensor_sub(out=var, in0=e2, in1=var)
        std = rowr(f"{nm}std")
        nc.scalar.activation(std, var, AF.Sqrt, bias=epsb)
        rstd = rowf(f"{nm}rstd")
        nc.vector.reciprocal(out=rstd, in_=std)
        stats = sb.tile([2, TOK], F32R, tag="sta", bufs=1, name=f"{nm}stats")
        if want_msr:
            msr = rowr(f"{nm}msr")
            nc.vector.tensor_mul(out=msr, in0=mu, in1=rstd)
            nc.sync.dma_start(out=stats[0:1, :], in_=msr)
            nc.sync.dma_start(out=stats[1:2, :], in_=d["oner"])
        else:
            nc.sync.dma_start(out=stats[0:1, :], in_=mu)
            nc.sync.dma_start(out=stats[1:2, :], in_=std)
        rcol = None
        if want_col:
            rcol = sb.tile([TT, NTT], F32, tag="rcol", bufs=1, name=f"{nm}rcol")
            pc = psA.tile([TT, NTT], F32, tag="mm", name=f"{nm}pcol")
            for m in range(NTT):
                nc.tensor.transpose(pc[:, m:m + 1],
                                    rstd.bitcast(F32)[:, m * TT:(m + 1) * TT], id1)
            nc.vector.tensor_copy(out=rcol, in_=pc)
        return stats, rstd, rcol

    # ---- embed: X = Wp.T @ xp + bp
    X = [None] * KT
    XP = []
    for k in range(KT):
        t = sb.tile([128, TOK], F32R, tag=f"xh{k}", name=f"XP{k}")
        nc.sync.dma_start(out=t, in_=d["xp"][k * 128:(k + 1) * 128, :])
        XP.append(t)

    def ev_embed(dp, ch, pm):
        if ch == 0:
            X[dp] = sb.tile([128, TOK], F32R, tag=f"x{dp}", name=f"X0_{dp}")
        nc.vector.tensor_copy(out=X[dp][:, chsl[ch]], in_=pm)

    dense_proj(d["wp"], XP, ev_embed, "emb")

    # ---- transformer layers
    for l in range(DEPTH):
        statsA, _rstd1, rcol = ln_stats(X, f"l{l}a", want_col=True, want_msr=False)

        # v-projection (activation-stationary -> token-major v), LN1 folded.
        VT = [None] * NTT
        wvf = []
        for k in range(KT):
            w = sb.tile([128, DIM], F32R, tag="wv", bufs=6, name=f"l{l}wv{k}")
            nc.sync.dma_start(out=w, in_=d["wv"][l, k * 128:(k + 1) * 128, :])
            wvf.append(w)
        wvb = sb.tile([2, DIM], F32R, tag="wvb", bufs=1, name=f"l{l}wvb")
        nc.sync.dma_start(out=wvb, in_=d["wv"][l, DIM:DIM + 2, :])
        # v-proj fused with attention per image (VT slots recycle promptly)
        OT = []
        for f in range(KT):
            o = sb.tile([128, TOK], F32R, tag=f"ot{f}", name=f"l{l}ot{f}")
            OT.append(o)
        for m in range(NTT):
            msl = slice(m * TT, (m + 1) * TT)
            VT[m] = sb.tile([TT, INNER], F32R, tag="vt", bufs=4, name=f"l{l}v{m}")
            for n2 in range(2):
                nsl = slice(n2 * 384, (n2 + 1) * 384)
                pv = mmtile([TT, 384], f"l{l}pv{m}_{n2}")
                for k in range(KT):
                    nc.tensor.matmul(pv, X[k][:, msl], wvf[k][:, nsl],
                                     start=(k == 0), stop=False)
                nc.tensor.matmul(pv, statsA[:, msl], wvb[:, nsl], start=False, stop=True)
                nc.vector.tensor_scalar_mul(VT[m][:, nsl], pv, rcol[:, m:m + 1])
            if m % 2 == 1:
                b = m // 2
                for f in range(KT):
                    po = mmtile([128, APAD], f"l{l}po{b}_{f}")
                    for h in range(2):
                        mm = 2 * b + h
                        nc.tensor.matmul(po, VT[mm][:, f * 128:(f + 1) * 128],
                                         ATT[:, mm * APAD:(mm + 1) * APAD],
                                         start=(h == 0), stop=(h == 1))
                    nc.scalar.activation(OT[f][:, b * N:(b + 1) * N], po[:, 0:N], AF.Copy)

        # out-projection + bout
        X2 = [None] * KT

        def ev_x2(dp, ch, pm, l=l):
            if ch == 0:
                X2[dp] = sb.tile([128, TOK], F32R, tag=f"x2_{dp}", name=f"l{l}x2_{dp}")
            nc.vector.tensor_copy(out=X2[dp][:, chsl[ch]], in_=pm)

        dense_proj(d["wo"][l], OT, ev_x2, f"l{l}o")

        # LN2 -> statsB rows [msr2, ones]; P2 broadcast; XH = X2 * rstd2
        statsB, rstd2, _ = ln_stats(X2, f"l{l}b", want_col=False, want_msr=True)
        nc.sync.dma_start(out=d["scrb"][l], in_=rstd2)
        P2 = sb.tile([128, TOK], F32, tag="p2", name=f"l{l}p2")
        nc.sync.dma_start(
            out=P2, in_=d["scrb"][l:l + 1, :].to_broadcast([128, TOK]))
        XH = []
        for k in range(KT):
            t = sb.tile([128, TOK], F32R, tag=f"xh{k}", name=f"l{l}xh{k}")
            nc.vector.tensor_mul(out=t, in0=X2[k], in1=P2)
            XH.append(t)

        # bff2 column
        bf2c = sb.tile([128, KT], F32, tag="bf2c", bufs=1, name=f"l{l}bf2c")
        nc.sync.dma_start(out=bf2c, in_=d["bf2"][l])

        # FFN: group-outer (weights streamed once), partial FFN2 accumulation
        # into the X tiles: X_next = sum_g partial_g + bff2 + X2.
        for dp in range(KT):
            X[dp] = sb.tile([128, TOK], F32R, tag=f"x{dp}", name=f"l{l}xn{dp}")
        for g in range(NG):
            gsl = slice(g * JG * 128, (g + 1) * JG * 128)
            w1g = []
            for k in range(KT):
                w = sb.tile([128, JG * 128], F32R, tag="wf1", bufs=12,
                            name=f"l{l}f1_{g}_{k}")
                nc.sync.dma_start(out=w, in_=d["wf1"][l, k * 128:(k + 1) * 128, gsl])
                w1g.append(w)
            w1b = sb.tile([2, JG * 128], F32R, tag="wf1b", bufs=2,
                          name=f"l{l}f1b{g}")
            nc.sync.dma_start(out=w1b, in_=d["wf1"][l, DIM:DIM + 2, gsl])
            w2g = []
            for jj in range(JG):
                j = g * JG + jj
                w2 = sb.tile([128, DIM], F32R, tag="wf2", bufs=2 * JG,
                             name=f"l{l}f2_{j}")
                nc.sync.dma_start(out=w2, in_=d["wf2"][l][:, j * DIM:(j + 1) * DIM])
                w2g.append(w2)
            for ch in range(NCH):
                acc = [psB.tile([128, CH], F32, tag="acc", name=f"l{l}a{g}_{ch}_{dp}")
                       for dp in range(KT)]
                for jj in range(JG):
                    j = g * JG + jj
                    jsl = slice(jj * 128, (jj + 1) * 128)
                    ph = psA.tile([128, CH], F32, tag="mm", name=f"l{l}ph{g}_{ch}_{jj}")
                    for k in range(KT):
                        nc.tensor.matmul(ph, w1g[k][:, jsl], XH[k][:, chsl[ch]],
                                         start=(k == 0), stop=False)
                    nc.tensor.matmul(ph, w1b[:, jsl], statsB[:, chsl[ch]],
                                     start=False, stop=True)
                    h1 = sb.tile([128, CH], F32R, tag="h1", bufs=3,
                                 name=f"l{l}h1_{g}_{ch}_{jj}")
                    nc.scalar.activation(h1, ph, AF.Gelu)
                    for dp in range(KT):
                        nc.tensor.matmul(acc[dp], w2g[jj][:, dp * 128:(dp + 1) * 128],
                                         h1, start=(jj == 0), stop=(jj == JG - 1))
                for dp in range(KT):
                    if g == 0:
                        nc.vector.scalar_tensor_tensor(
                            out=X[dp][:, chsl[ch]], in0=acc[dp],
                            scalar=bf2c[:, dp:dp + 1],
                            in1=X2[dp][:, chsl[ch]], op0=ALU.add, op1=ALU.add)
                    else:
                        nc.vector.tensor_add(
                            out=X[dp][:, chsl[ch]], in0=X[dp][:, chsl[ch]],
                            in1=acc[dp])

    # ---- head: out = Wep.T @ x + bep
    def ev_out(dp, ch, pm):
        t = sb.tile([128, CH], F32, tag="scr", bufs=3, name=f"hd{dp}_{ch}")
        nc.vector.tensor_copy(out=t, in_=pm)
        nc.sync.dma_start(out=d["out"][dp * 128:(dp + 1) * 128, chsl[ch]], in_=t)

    dense_proj(d["we"], X, ev_out, "hd")


# ------------------------------------------------------------------ host prep
def _softplus(x):
    return np.logaddexp(x, 0.0)


def _attention(A_noise, b_noise, A_mean, b_mean, A_std, b_std, band_width):
    m, n = np.meshgrid(np.arange(G), np.arange(G))
    Xc = np.stack((m, n), axis=-1).reshape(-1, 2).astype(np.float64)
    A = A_mean.astype(np.float64) + _softplus(A_std.astype(np.float64)) * A_noise.astype(np.float64)
    bt = b_mean.astype(np.float64) + _softplus(b_std.astype(np.float64)) * b_noise.astype(np.float64)
    Xt = np.einsum('pd,bdo->bpo', Xc, A) + bt[:, None, :]
    diff = Xt[:, None, :, :] - Xc[None, :, None, :]
    dots = -np.sum(diff * diff, axis=-1) / float(band_width)
    dots -= dots.max(axis=-1, keepdims=True)
    e = np.exp(dots)
    attn = e / e.sum(axis=-1, keepdims=True)
    return attn.astype(np.float32)            # (B, N, N)


def _prep(inputs):
    f = np.float32
    w = {}
    Wp = inputs["Wp"].astype(f); bp = inputs["bp"].astype(f)
    w["wp"] = np.ascontiguousarray(np.vstack([Wp, bp[None, :]]))
    Wep = inputs["Wep"].astype(f); bep = inputs["bep"].astype(f)
    w["we"] = np.ascontiguousarray(np.vstack([Wep, bep[None, :]]))
    wv = np.zeros((DEPTH, DIM + 2, DIM), f)
    wo = np.zeros((DEPTH, DIM + 1, DIM), f)
    wf1 = np.zeros((DEPTH, DIM + 2, MLP), f)
    wf2 = np.zeros((DEPTH, 128, KM * DIM), f)
    bf2 = np.zeros((DEPTH, 128, KT), f)
    for l in range(DEPTH):
        g1 = inputs["ln1_g"][l].astype(f); b1 = inputs["ln1_b"][l].astype(f)
        Wv = inputs["Wqkv"][l][:, 2 * INNER:].astype(f)
        wv[l, :DIM] = g1[:, None] * Wv
        wv[l, DIM] = -(g1 @ Wv)
        wv[l, DIM + 1] = b1 @ Wv
        wo[l, :DIM] = inputs["Wout"][l].astype(f)
        wo[l, DIM] = inputs["bout"][l].astype(f)
        g2 = inputs["ln2_g"][l].astype(f); b2 = inputs["ln2_b"][l].astype(f)
        Wf1 = inputs["Wff1"][l].astype(f)
        wf1[l, :DIM] = g2[:, None] * Wf1
        wf1[l, DIM] = -(g2 @ Wf1)
        wf1[l, DIM + 1] = b2 @ Wf1 + inputs["bff1"][l].astype(f)
        wf2[l] = inputs["Wff2"][l].astype(f).reshape(KM, 128, DIM).transpose(1, 0, 2).reshape(128, KM * DIM)
        bf2[l] = inputs["bff2"][l].astype(f).reshape(KT, 128).T
    w["wv"] = wv; w["wo"] = wo; w["wf1"] = wf1; w["wf2"] = wf2; w["bf2"] = bf2
    w["onec"] = np.ones((128, 1), f)
    w["oner"] = np.ones((1, TOK), f)
    return w


def _prep_percore(inputs):
    f = np.float32
    img = inputs["img"].astype(f)
    patches = img.reshape(B, C, G, PCH, G, PCH).transpose(0, 2, 4, 3, 5, 1).reshape(B, N, PD)
    attn = _attention(inputs["A_noise"], inputs["b_noise"], inputs["A_mean"],
                      inputs["b_mean"], inputs["A_std"], inputs["b_std"],
                      inputs["band_width"])
    per = []
    for c in range(NCORES):
        pc = patches[c * BPC:(c + 1) * BPC].reshape(TOK, PD)
        xp = np.ascontiguousarray(pc.T)
        ac = attn[c * BPC:(c + 1) * BPC]                      # (4, N, N) [b, i, j]
        at = np.zeros((TT, NTT, APAD), f)
        at[:, :, :N] = ac.transpose(0, 2, 1).reshape(BPC, 2, TT, N).transpose(2, 0, 1, 3).reshape(TT, NTT, N)
        per.append({"xp": xp, "att": np.ascontiguousarray(at.reshape(TT, NTT * APAD))})
    return per


def _unpack(results):
    outs = []
    for c in range(NCORES):
        o = results[c]["out"]                                # (PD, TOK)
        x = o.T.reshape(BPC, G, G, PCH, PCH, C).transpose(0, 5, 1, 3, 2, 4)
        outs.append(x.reshape(BPC, C, IMG, IMG))
    return np.concatenate(outs, axis=0)


# ------------------------------------------------------------------- kernel()
def kernel(**inputs) -> np.ndarray:
    if "nc" not in _STATE:
        _STATE["nc"] = _build()
    if "w" not in _STATE:
        _STATE["w"] = _prep(inputs)
    per = _prep_percore(inputs)
    in_maps = [{**per[c], **_STATE["w"]} for c in range(NCORES)]
    res = run_bass_kernel_spmd(_STATE["nc"], in_maps, core_ids=list(range(NCORES)))
    return _unpack(res.results)


# revision 19
# speedup vs baseline: 1.3006x; 1.0586x over previous
"""Trainium2 Bass kernel for nn_ExplicitGeometricAugmentor.

Data-parallel over batch: 32 images -> 8 cores x 4 images. No collectives.

Per-core layout: feature-major activations [dim, tok] (tok = 4*196 = 784).
All matmuls fp32r (full PE rate at moving-free >= 256). LayerNorm is folded
into the adjacent matmuls via augmented contraction rows; only v (not q,k)
of the qkv projection is computed since the RBF attention ignores q,k.
"""
import math
import numpy as np

import concourse.mybir as mybir
import concourse.tile as tile
from concourse import bacc
from concourse.bass_utils import run_bass_kernel_spmd

F32 = mybir.dt.float32
F32R = mybir.dt.float32r
AF = mybir.ActivationFunctionType
ALU = mybir.AluOpType

IMG = 224; PCH = 16; C = 3
G = IMG // PCH; N = G * G          # 14, 196
DIM = 768; DEPTH = 6; MLP = 3072
INNER = 768; PD = 768
B = 32
NCORES = 8
BPC = B // NCORES                  # 4 images per core
TOK = BPC * N                      # 784 tokens per core
TT = 98                            # token tile (196 = 2*98 -> image == 2 tiles)
NTT = TOK // TT                    # 8 token tiles
KT = DIM // 128                    # 6 feature k-tiles
KM = MLP // 128                    # 24 mlp k-tiles
CH = 392                           # token chunk (free dim of most matmuls)
NCH = TOK // CH                    # 2
APAD = 256                         # attention moving-dim pad (fp32r full rate)
EPS = 1e-5
JG = 4                             # ffn j-group size (weight residency granularity)
NG = KM // JG                      # 6 groups

_STATE: dict = {}


# ----------------------------------------------------------------- bass build
def _build():
    nc = bacc.Bacc("TRN2", debug=False, target_bir_lowering=False)
    d = {}
    d["xp"] = nc.dram_tensor("xp", (DIM, TOK), F32R, kind="ExternalInput").ap()
    d["att"] = nc.dram_tensor("att", (TT, NTT * APAD), F32R, kind="ExternalInput").ap()
    d["wp"] = nc.dram_tensor("wp", (DIM + 1, DIM), F32R, kind="ExternalInput").ap()
    d["wv"] = nc.dram_tensor("wv", (DEPTH, DIM + 2, DIM), F32R, kind="ExternalInput").ap()
    d["wo"] = nc.dram_tensor("wo", (DEPTH, DIM + 1, DIM), F32R, kind="ExternalInput").ap()
    d["wf1"] = nc.dram_tensor("wf1", (DEPTH, DIM + 2, MLP), F32R, kind="ExternalInput").ap()
    d["wf2"] = nc.dram_tensor("wf2", (DEPTH, 128, KM * DIM), F32R, kind="ExternalInput").ap()
    d["bf2"] = nc.dram_tensor("bf2", (DEPTH, 128, KT), F32, kind="ExternalInput").ap()
    d["we"] = nc.dram_tensor("we", (DIM + 1, PD), F32R, kind="ExternalInput").ap()
    d["onec"] = nc.dram_tensor("onec", (128, 1), F32R, kind="ExternalInput").ap()
    d["oner"] = nc.dram_tensor("oner", (1, TOK), F32R, kind="ExternalInput").ap()
    d["out"] = nc.dram_tensor("out", (PD, TOK), F32, kind="ExternalOutput").ap()
    d["scrb"] = nc.dram_tensor("scrb", (DEPTH, TOK), F32, kind="Internal").ap()

    with tile.TileContext(nc) as tc:
        with tc.tile_pool(name="sb", bufs=1) as sb, \
             tc.tile_pool(name="psA", bufs=2, space="PSUM") as psA, \
             tc.tile_pool(name="psB", bufs=6, space="PSUM") as psB:
            _emit(nc, tc, d, sb, psA, psB)
    nc.compile()
    return nc


def _emit(nc, tc, d, sb, psA, psB):
    chsl = [slice(ch * CH, (ch + 1) * CH) for ch in range(NCH)]
    _mmctr = [0]

    def mmtile(shape, name):
        # round-robin psum slots: 2 in psA ("mm") + 6 in psB ("acc") = 8 banks
        _mmctr[0] += 1
        if _mmctr[0] % 4 == 0:
            return psA.tile(shape, F32, tag="mm", name=name)
        return psB.tile(shape, F32, tag="acc", name=name)

    # ---- persistent constants
    ATT = sb.tile([TT, NTT * APAD], F32R, tag="attc", name="ATT")
    nc.sync.dma_start(out=ATT, in_=d["att"])
    ones_col = sb.tile([128, 1], F32R, tag="onec", name="ones_col")
    nc.sync.dma_start(out=ones_col, in_=d["onec"])
    oner_sb = sb.tile([1, TOK], F32R, tag="onerr", name="oner_sb")
    nc.sync.dma_start(out=oner_sb, in_=d["oner"])
    id1 = sb.tile([1, 1], F32, tag="id1", name="id1")
    nc.vector.memset(id1, 1.0)
    epsb = sb.tile([1, 1], F32, tag="epsb", name="epsb")
    nc.vector.memset(epsb, EPS)

    def rowf(nm):
        return sb.tile([1, TOK], F32, tag="rwf", bufs=3, name=nm)

    def rowr(nm):
        return sb.tile([1, TOK], F32R, tag="rwr", bufs=3, name=nm)

    # ---- dense projection out[dp 128-tiles, tok] = W.T @ src (+ bias row via ones aug)
    # wsrc rows: [0:DIM]=W, row DIM = bias. src: list of 6 [128, TOK] f32r tiles.
    def dense_proj(wsrc, src, evict, nm):
        for dp in range(KT):
            wt = []
            for k in range(KT):
                w = sb.tile([128, 128], F32R, tag="wod", bufs=8, name=f"{nm}w{dp}_{k}")
                nc.sync.dma_start(out=w, in_=wsrc[k * 128:(k + 1) * 128,
                                                 dp * 128:(dp + 1) * 128])
                wt.append(w)
            wb = sb.tile([1, 128], F32R, tag="wob", bufs=2, name=f"{nm}b{dp}")
            nc.sync.dma_start(out=wb, in_=wsrc[DIM:DIM + 1, dp * 128:(dp + 1) * 128])
            for ch in range(NCH):
                pm = mmtile([128, CH], f"{nm}p{dp}_{ch}")
                for k in range(KT):
                    nc.tensor.matmul(pm, wt[k], src[k][:, chsl[ch]],
                                     start=(k == 0), stop=False)
                nc.tensor.matmul(pm, wb, oner_sb[:, chsl[ch]], start=False, stop=True)
                evict(dp, ch, pm)

    # ---- LN stats for 6 [128, TOK] f32r tiles -> (statsA[2,TOK] f32r rows, rstd f32 row)
    def ln_stats(src, nm, want_col, want_msr):
        mu = rowr(f"{nm}mu")
        e2 = rowf(f"{nm}e2")
        var = rowf(f"{nm}var")
        std = rowr(f"{nm}std")
        rstd = rowf(f"{nm}rstd")
        msr = rowr(f"{nm}msr") if want_msr else None
        stats = sb.tile([2, TOK], F32R, tag="sta", bufs=1, name=f"{nm}stats")
        for ch in range(NCH):
            c = chsl[ch]
            pS = mmtile([1, CH], f"{nm}pS{ch}")
            pQ = mmtile([1, CH], f"{nm}pQ{ch}")
            for k in range(KT):
                nc.tensor.matmul(pS, ones_col, src[k][:, c],
                                 start=(k == 0), stop=(k == KT - 1))
            for k in range(KT):
                s = sb.tile([128, CH], F32R, tag="scr", bufs=3, name=f"{nm}sq{ch}_{k}")
                nc.vector.tensor_mul(out=s, in0=src[k][:, c], in1=src[k][:, c])
                nc.tensor.matmul(pQ, ones_col, s,
                                 start=(k == 0), stop=(k == KT - 1))
            nc.vector.tensor_scalar_mul(mu[:, c], pS, 1.0 / DIM)
            nc.vector.tensor_scalar_mul(e2[:, c], pQ, 1.0 / DIM)
            nc.vector.tensor_mul(out=var[:, c], in0=mu[:, c], in1=mu[:, c])
            nc.vector.tensor_sub(out=var[:, c], in0=e2[:, c], in1=var[:, c])
            nc.scalar.activation(std[:, c], var[:, c], AF.Sqrt, bias=epsb)
            nc.vector.reciprocal(out=rstd[:, c], in_=std[:, c])
            if want_msr:
                nc.vector.tensor_mul(out=msr[:, c], in0=mu[:, c], in1=rstd[:, c])
                nc.sync.dma_start(out=stats[0:1, c], in_=msr[:, c])
                nc.sync.dma_start(out=stats[1:2, c], in_=d["oner"][:, c])
            else:
                nc.sync.dma_start(out=stats[0:1, c], in_=mu[:, c])
                nc.sync.dma_start(out=stats[1:2, c], in_=std[:, c])
        rcol = None
        if want_col:
            rcol = sb.tile([TT, NTT], F32, tag="rcol", bufs=1, name=f"{nm}rcol")
            half = NTT // NCH
            for ch in range(NCH):
                pc = psA.tile([TT, half], F32, tag="mm", name=f"{nm}pcol{ch}")
                for mm_ in range(half):
                    m = ch * half + mm_
                    nc.tensor.transpose(pc[:, mm_:mm_ + 1],
                                        rstd.bitcast(F32)[:, m * TT:(m + 1) * TT], id1)
                nc.vector.tensor_copy(out=rcol[:, ch * half:(ch + 1) * half], in_=pc)
        return stats, rstd, rcol

    # ---- embed: X = Wp.T @ xp + bp
    X = [None] * KT
    XP = []
    for k in range(KT):
        t = sb.tile([128, TOK], F32R, tag=f"xh{k}", name=f"XP{k}")
        nc.sync.dma_start(out=t, in_=d["xp"][k * 128:(k + 1) * 128, :])
        XP.append(t)

    def ev_embed(dp, ch, pm):
        if ch == 0:
            X[dp] = sb.tile([128, TOK], F32R, tag=f"x{dp}", name=f"X0_{dp}")
        nc.vector.tensor_copy(out=X[dp][:, chsl[ch]], in_=pm)

    dense_proj(d["wp"], XP, ev_embed, "emb")

    # ---- transformer layers
    for l in range(DEPTH):
        statsA, _rstd1, rcol = ln_stats(X, f"l{l}a", want_col=True, want_msr=False)

        # v-projection (activation-stationary -> token-major v), LN1 folded.
        VT = [None] * NTT
        wvf = []
        for k in range(KT):
            w = sb.tile([128, DIM], F32R, tag="wv", bufs=7, name=f"l{l}wv{k}")
            nc.sync.dma_start(out=w, in_=d["wv"][l, k * 128:(k + 1) * 128, :])
            wvf.append(w)
        wvb = sb.tile([2, DIM], F32R, tag="wvb", bufs=1, name=f"l{l}wvb")
        nc.sync.dma_start(out=wvb, in_=d["wv"][l, DIM:DIM + 2, :])
        # v-proj fused with attention per image (VT slots recycle promptly)
        OT = []
        for f in range(KT):
            o = sb.tile([128, TOK], F32R, tag=f"ot{f}", name=f"l{l}ot{f}")
            OT.append(o)
        for m in range(NTT):
            msl = slice(m * TT, (m + 1) * TT)
            VT[m] = sb.tile([TT, INNER], F32R, tag="vt", bufs=4, name=f"l{l}v{m}")
            for n2 in range(2):
                nsl = slice(n2 * 384, (n2 + 1) * 384)
                pv = mmtile([TT, 384], f"l{l}pv{m}_{n2}")
                for k in range(KT):
                    nc.tensor.matmul(pv, X[k][:, msl], wvf[k][:, nsl],
                                     start=(k == 0), stop=False)
                nc.tensor.matmul(pv, statsA[:, msl], wvb[:, nsl], start=False, stop=True)
                nc.vector.tensor_scalar_mul(VT[m][:, nsl], pv, rcol[:, m:m + 1])
            if m % 2 == 1:
                b = m // 2
                for f in range(KT):
                    po = mmtile([128, APAD], f"l{l}po{b}_{f}")
                    for h in range(2):
                        mm = 2 * b + h
                        nc.tensor.matmul(po, VT[mm][:, f * 128:(f + 1) * 128],
                                         ATT[:, mm * APAD:(mm + 1) * APAD],
                                         start=(h == 0), stop=(h == 1))
                    nc.scalar.activation(OT[f][:, b * N:(b + 1) * N], po[:, 0:N], AF.Copy)

        # out-projection + bout
        X2 = [None] * KT

        def ev_x2(dp, ch, pm, l=l):
            if ch == 0:
                X2[dp] = sb.tile([128, TOK], F32R, tag=f"x2_{dp}", name=f"l{l}x2_{dp}")
            nc.vector.tensor_copy(out=X2[dp][:, chsl[ch]], in_=pm)

        dense_proj(d["wo"][l], OT, ev_x2, f"l{l}o")

        # LN2 -> statsB rows [msr2, ones]; P2 broadcast; XH = X2 * rstd2
        statsB, rstd2, _ = ln_stats(X2, f"l{l}b", want_col=False, want_msr=True)
        P2 = sb.tile([128, TOK], F32, tag="p2", name=f"l{l}p2")
        XH = [sb.tile([128, TOK], F32R, tag=f"xh{k}", name=f"l{l}xh{k}")
              for k in range(KT)]
        for ch in range(NCH):
            c = chsl[ch]
            nc.sync.dma_start(out=d["scrb"][l, c], in_=rstd2[:, c])
            nc.sync.dma_start(out=P2[:, c],
                              in_=d["scrb"][l:l + 1, c].to_broadcast([128, CH]))
            for k in range(KT):
                nc.vector.tensor_mul(out=XH[k][:, c], in0=X2[k][:, c], in1=P2[:, c])

        # bff2 column
        bf2c = sb.tile([128, KT], F32, tag="bf2c", bufs=1, name=f"l{l}bf2c")
        nc.sync.dma_start(out=bf2c, in_=d["bf2"][l])

        # FFN: group-outer (weights streamed once), partial FFN2 accumulation
        # into the X tiles: X_next = sum_g partial_g + bff2 + X2.
        for dp in range(KT):
            X[dp] = sb.tile([128, TOK], F32R, tag=f"x{dp}", name=f"l{l}xn{dp}")
        from collections import deque
        pend = deque()  # (acc, jj, h1) pending FFN2 emissions, depth 2

        def flush_pend(n):
            while len(pend) > n:
                acc_, pj, (ph1, pw2) = pend.popleft()
                for dp in range(KT):
                    nc.tensor.matmul(acc_[dp], pw2[:, dp * 128:(dp + 1) * 128],
                                     ph1, start=(pj == 0), stop=(pj == JG - 1))
        for g in range(NG):
            gsl = slice(g * JG * 128, (g + 1) * JG * 128)
            w1g = []
            for k in range(KT):
                w = sb.tile([128, JG * 128], F32R, tag="wf1", bufs=10,
                            name=f"l{l}f1_{g}_{k}")
                nc.sync.dma_start(out=w, in_=d["wf1"][l, k * 128:(k + 1) * 128, gsl])
                w1g.append(w)
            w1b = sb.tile([2, JG * 128], F32R, tag="wf1b", bufs=2,
                          name=f"l{l}f1b{g}")
            nc.sync.dma_start(out=w1b, in_=d["wf1"][l, DIM:DIM + 2, gsl])
            w2g = []
            for jj in range(JG):
                j = g * JG + jj
                w2 = sb.tile([128, DIM], F32R, tag="wf2", bufs=7,
                             name=f"l{l}f2_{j}")
                nc.sync.dma_start(out=w2, in_=d["wf2"][l][:, j * DIM:(j + 1) * DIM])
                w2g.append(w2)
            for ch in range(NCH):
                acc = [psB.tile([128, CH], F32, tag="acc", name=f"l{l}a{g}_{ch}_{dp}")
                       for dp in range(KT)]
                # software-pipelined emission with a 2-deep global pend
                # queue: FFN2(j) is emitted two FFN1 groups later so the
                # in-order PE stream never waits on gelu.
                for jj in range(JG):
                    j = g * JG + jj
                    jsl = slice(jj * 128, (jj + 1) * 128)
                    ph = psA.tile([128, CH], F32, tag="mm", name=f"l{l}ph{g}_{ch}_{jj}")
                    for k in range(KT):
                        nc.tensor.matmul(ph, w1g[k][:, jsl], XH[k][:, chsl[ch]],
                                         start=(k == 0), stop=False)
                    nc.tensor.matmul(ph, w1b[:, jsl], statsB[:, chsl[ch]],
                                     start=False, stop=True)
                    flush_pend(1)
                    h1 = sb.tile([128, CH], F32R, tag="h1", bufs=4,
                                 name=f"l{l}h1_{g}_{ch}_{jj}")
                    nc.scalar.activation(h1, ph, AF.Gelu)
                    pend.append((acc, jj, (h1, w2g[jj])))
                flush_pend(0)
                for dp in range(KT):
                    if g == 0:
                        nc.vector.scalar_tensor_tensor(
                            out=X[dp][:, chsl[ch]], in0=acc[dp],
                            scalar=bf2c[:, dp:dp + 1],
                            in1=X2[dp][:, chsl[ch]], op0=ALU.add, op1=ALU.add)
                    else:
                        nc.vector.tensor_add(
                            out=X[dp][:, chsl[ch]], in0=X[dp][:, chsl[ch]],
                            in1=acc[dp])

    # ---- head: out = Wep.T @ x + bep
    def ev_out(dp, ch, pm):
        t = sb.tile([128, CH], F32, tag="scr", bufs=3, name=f"hd{dp}_{ch}")
        nc.vector.tensor_copy(out=t, in_=pm)
        nc.sync.dma_start(out=d["out"][dp * 128:(dp + 1) * 128, chsl[ch]], in_=t)

    dense_proj(d["we"], X, ev_out, "hd")


# ------------------------------------------------------------------ host prep
def _softplus(x):
    return np.logaddexp(x, 0.0)


def _attention(A_noise, b_noise, A_mean, b_mean, A_std, b_std, band_width):
    m, n = np.meshgrid(np.arange(G), np.arange(G))
    Xc = np.stack((m, n), axis=-1).reshape(-1, 2).astype(np.float64)
    A = A_mean.astype(np.float64) + _softplus(A_std.astype(np.float64)) * A_noise.astype(np.float64)
    bt = b_mean.astype(np.float64) + _softplus(b_std.astype(np.float64)) * b_noise.astype(np.float64)
    Xt = np.einsum('pd,bdo->bpo', Xc, A) + bt[:, None, :]
    diff = Xt[:, None, :, :] - Xc[None, :, None, :]
    dots = -np.sum(diff * diff, axis=-1) / float(band_width)
    dots -= dots.max(axis=-1, keepdims=True)
    e = np.exp(dots)
    attn = e / e.sum(axis=-1, keepdims=True)
    return attn.astype(np.float32)            # (B, N, N)


def _prep(inputs):
    f = np.float32
    w = {}
    Wp = inputs["Wp"].astype(f); bp = inputs["bp"].astype(f)
    w["wp"] = np.ascontiguousarray(np.vstack([Wp, bp[None, :]]))
    Wep = inputs["Wep"].astype(f); bep = inputs["bep"].astype(f)
    w["we"] = np.ascontiguousarray(np.vstack([Wep, bep[None, :]]))
    wv = np.zeros((DEPTH, DIM + 2, DIM), f)
    wo = np.zeros((DEPTH, DIM + 1, DIM), f)
    wf1 = np.zeros((DEPTH, DIM + 2, MLP), f)
    wf2 = np.zeros((DEPTH, 128, KM * DIM), f)
    bf2 = np.zeros((DEPTH, 128, KT), f)
    for l in range(DEPTH):
        g1 = inputs["ln1_g"][l].astype(f); b1 = inputs["ln1_b"][l].astype(f)
        Wv = inputs["Wqkv"][l][:, 2 * INNER:].astype(f)
        wv[l, :DIM] = g1[:, None] * Wv
        wv[l, DIM] = -(g1 @ Wv)
        wv[l, DIM + 1] = b1 @ Wv
        wo[l, :DIM] = inputs["Wout"][l].astype(f)
        wo[l, DIM] = inputs["bout"][l].astype(f)
        g2 = inputs["ln2_g"][l].astype(f); b2 = inputs["ln2_b"][l].astype(f)
        Wf1 = inputs["Wff1"][l].astype(f)
        wf1[l, :DIM] = g2[:, None] * Wf1
        wf1[l, DIM] = -(g2 @ Wf1)
        wf1[l, DIM + 1] = b2 @ Wf1 + inputs["bff1"][l].astype(f)
        wf2[l] = inputs["Wff2"][l].astype(f).reshape(KM, 128, DIM).transpose(1, 0, 2).reshape(128, KM * DIM)
        bf2[l] = inputs["bff2"][l].astype(f).reshape(KT, 128).T
    w["wv"] = wv; w["wo"] = wo; w["wf1"] = wf1; w["wf2"] = wf2; w["bf2"] = bf2
    w["onec"] = np.ones((128, 1), f)
    w["oner"] = np.ones((1, TOK), f)
    return w


def _prep_percore(inputs):
    f = np.float32
    img = inputs["img"].astype(f)
    patches = img.reshape(B, C, G, PCH, G, PCH).transpose(0, 2, 4, 3, 5, 1).reshape(B, N, PD)
    attn = _attention(inputs["A_noise"], inputs["b_noise"], inputs["A_mean"],
                      inputs["b_mean"], inputs["A_std"], inputs["b_std"],
                      inputs["band_width"])
    per = []
    for c in range(NCORES):
        pc = patches[c * BPC:(c + 1) * BPC].reshape(TOK, PD)
        xp = np.ascontiguousarray(pc.T)
        ac = attn[c * BPC:(c + 1) * BPC]                      # (4, N, N) [b, i, j]
        at = np.zeros((TT, NTT, APAD), f)
        at[:, :, :N] = ac.transpose(0, 2, 1).reshape(BPC, 2, TT, N).transpose(2, 0, 1, 3).reshape(TT, NTT, N)
        per.append({"xp": xp, "att": np.ascontiguousarray(at.reshape(TT, NTT * APAD))})
    return per


def _unpack(results):
    outs = []
    for c in range(NCORES):
        o = results[c]["out"]                                # (PD, TOK)
        x = o.T.reshape(BPC, G, G, PCH, PCH, C).transpose(0, 5, 1, 3, 2, 4)
        outs.append(x.reshape(BPC, C, IMG, IMG))
    return np.concatenate(outs, axis=0)


# ------------------------------------------------------------------- kernel()
def kernel(**inputs) -> np.ndarray:
    if "nc" not in _STATE:
        _STATE["nc"] = _build()
    if "w" not in _STATE:
        _STATE["w"] = _prep(inputs)
    per = _prep_percore(inputs)
    in_maps = [{**per[c], **_STATE["w"]} for c in range(NCORES)]
    res = run_bass_kernel_spmd(_STATE["nc"], in_maps, core_ids=list(range(NCORES)))
    return _unpack(res.results)
